# revision 6
# baseline (speedup 1.0000x reference)
"""Cosine-similarity self-attention (Cos_Attn) on 8 Trainium2 NeuronCores.

Reference math (x: [C=512, W=64, H=64] fp32, N = W*H = 4096):
    q = x.reshape(C, N).T                  # [N, C]
    energy = q @ q.T                       # [N, N]
    cos    = energy / (|q_i| |q_j|)
    out    = softmax(cos, axis=-1)[None]   # [1, N, N]

Sharding: N query rows split across 8 cores (512 rows each). One shared
program; per-core asymmetry is handled by ROTATING the input data so that
device-block 0 is always the core's own query block. Host un-rotates the
output columns.

v4 design (per core). Error budget 2e-2 rel-Frobenius; fp8e4 keys give
~0.4% after the 512-term dot-product averaging, bf16 I/O ~0.4%:
  - input x cast to fp8e4 on host, block-major [NB=8, P=128, KO=4, CB]
    (2MB per core, 128 x 2KB descriptors per block DMA).
  - per block: DVE squares (fp8, 2x_2p) -> PE DoubleRow ones-matmul
    column sums (PSUM f32) -> DVE reciprocal_approx_fast -> ACT
    Sqrt(scale=64) -> rn = 8/|q| bf16 -> DVE normalize xn = x*rn (fp8,
    values ~8x for e4m3 range; the 1/64 undone in the exp scale).
  - queries are xn block 0 (column slice of the keys).
  - energy: PE fp8 DoubleRow (0.5 cyc/row), 4-bank PSUM groups
    [P, 4, CB], two k-pair matmuls per bank; ns shares the same PSUM
    pool tag so 2x4 banks double-buffer cleanly (PSUM is exactly 8).
  - softmax: ACT Exp(scale=1/64) straight out of PSUM, 2048 wide, bf16
    out, accum_out row sums; max-subtraction skipped (cos in [-1,1]).
  - ACT table discipline: ALL sqrts strictly before ALL exps -> exactly
    2 ACT_TABLE_LOADs (compiler picks tables greedily; any interleave
    costs 1.28us per switch). Copy is in every set -> free.
  - m-major energy order, NG=2 groups per row tile: each row tile's
    1/rowsum tail (ACT Copy rr_row + DVE bf16 multiply + out DMA)
    follows its second exp, so tails overlap the remaining exp chain
    instead of bunching at the end.
  - output bf16 [MT, P, N], host upcasts to f32.
"""

import numpy as np

_NCORES = 8
_P = 128

# set by the test harness only; the grading path keeps these defaults
TRACE = False
TRACE_CORES = None
LAST_RESULT = None

_built = None  # (nc, C, N)

GB = 4            # blocks per energy group (PSUM banks per tile)
WARMUP_MM = 6     # junk matmuls to ramp the PE p-state during DMA wait
PT_BUFS = 2


def _build(C, N):
    from contextlib import ExitStack

    import concourse.tile as tile
    from concourse import bacc, mybir

    f32 = mybir.dt.float32
    bf16 = mybir.dt.bfloat16
    fp8 = mybir.dt.float8e4
    AF = mybir.ActivationFunctionType
    AX = mybir.AxisListType
    OP = mybir.AluOpType
    DR = mybir.MatmulPerfMode.DoubleRow

    P = _P
    KO = C // P              # contraction subtiles (4)
    KP = KO // 2             # DoubleRow k-pairs (2)
    CB = 512                 # column block = one PSUM bank of f32
    NB = N // CB             # 8 column blocks
    MT = (N // _NCORES) // P # 4 query row tiles per core
    NG = NB // GB            # 2 energy groups per row tile

    nc = bacc.Bacc("TRN2", target_bir_lowering=False, debug=False)
    x_d = nc.dram_tensor("x", [NB, P, KO, CB], fp8, kind="ExternalInput")
    out_d = nc.dram_tensor("out", [MT, P, N], bf16, kind="ExternalOutput")

    with tile.TileContext(nc) as tc, ExitStack() as ctx:
        persist = ctx.enter_context(tc.tile_pool(name="persist", bufs=1))
        temps = ctx.enter_context(tc.tile_pool(name="temps", bufs=3))
        psum = ctx.enter_context(tc.tile_pool(name="psum", bufs=2, space="PSUM"))

        xn = persist.tile([P, KO, N], fp8)       # normalized keys (and queries)
        e = persist.tile([P, MT, N], bf16)       # exp(cos); scaled in place
        rn = persist.tile([P, N], bf16)          # 8/|q_j| replicated on parts
        sums = persist.tile([P, MT, NG], f32)    # per-(m, g) exp row sums
        rs = persist.tile([P, MT], f32)
        rr = persist.tile([P, MT], f32)
        ones2 = persist.tile([P, 2, P], fp8)     # DoubleRow lhsT of ones
        ones_row = persist.tile([P, CB], f32)
        nc.vector.memset(ones2[:], 1.0)
        nc.vector.memset(ones_row[:], 1.0)

        xr_tiles = {}
        r1s = {}

        def dma_in(b):
            xr = temps.tile([P, KO, CB], fp8, tag="xr", name="xr", bufs=5)
            nc.sync.dma_start(xr[:], x_d.ap()[b])
            xr_tiles[b] = xr

        def pt_alloc(name):
            return psum.tile([P, GB, CB], f32, tag="pt", name=name,
                             bufs=PT_BUFS)

        def warmup_pe():
            junk = pt_alloc("junk")
            for i in range(WARMUP_MM):
                nc.tensor.matmul(junk[:, 0, 0:P], lhsT=ones2[:, 0, :],
                                 rhs=ones2[:, 0, :],
                                 start=(i == 0), stop=(i == WARMUP_MM - 1))

        def norm_pre(b):
            """squares -> DoubleRow colsum matmul -> approx reciprocal."""
            xsq = temps.tile([P, KO, CB], fp8, tag="xsq", name="xsq", bufs=2)
            nc.vector.tensor_mul(xsq[:], xr_tiles[b][:], xr_tiles[b][:])
            ns = pt_alloc("ns")
            for kp in range(KP):
                nc.tensor.matmul(
                    ns[:, 0, :], lhsT=ones2[:],
                    rhs=xsq[:, 2 * kp:2 * kp + 2, :],
                    start=(kp == 0), stop=(kp == KP - 1), perf_mode=DR,
                )
            r1 = temps.tile([P, CB], f32, tag="r1", name="r1", bufs=2)
            nc.vector.reciprocal_approx_fast(r1[:], ns[:, 0, :])
            r1s[b] = r1

        def block_sqrt(b):
            cs = slice(b * CB, (b + 1) * CB)
            # rn = sqrt(64 / ns) = 8 / |q|; the 8x on keys gives the fp8
            # mantissa more range, undone by the 1/64 exp scale
            nc.scalar.activation(rn[:, cs], r1s.pop(b)[:], AF.Sqrt,
                                 scale=64.0)

        def block_normalize(b):
            cs = slice(b * CB, (b + 1) * CB)
            rn_b = rn[:, None, cs].to_broadcast([P, KO, CB])
            nc.vector.tensor_mul(xn[:, :, cs], xr_tiles.pop(b)[:], rn_b)

        def energy_group(m, g):
            ms = slice(m * P, (m + 1) * P)
            pt = pt_alloc("pt")
            for kp in range(KP):
                ks = slice(2 * kp, 2 * kp + 2)
                for j in range(GB):
                    b = g * GB + j
                    cs = slice(b * CB, (b + 1) * CB)
                    nc.tensor.matmul(
                        pt[:, j, :],
                        lhsT=xn[:, ks, ms],  # queries = block-0 cols of xn
                        rhs=xn[:, ks, cs],
                        start=(kp == 0), stop=(kp == KP - 1), perf_mode=DR,
                    )
            return pt

        def exp_group(m, g, pt):
            gs = slice(g * GB * CB, (g + 1) * GB * CB)
            nc.scalar.activation(
                e[:, m, gs], pt[:].rearrange("p a b -> p (a b)"), AF.Exp,
                scale=1.0 / 64.0, accum_out=sums[:, m, g:g + 1],
            )

        def tail(m):
            """row scale + output DMA for row tile m."""
            nc.vector.tensor_reduce(
                rs[:, m:m + 1], sums[:, m, :], axis=AX.X, op=OP.add)
            nc.vector.reciprocal(rr[:, m:m + 1], rs[:, m:m + 1])
            rr_row = temps.tile([P, CB], bf16, tag="rr_row", name="rr_row",
                                bufs=2)
            nc.scalar.activation(rr_row[:], ones_row[:], AF.Copy,
                                 scale=rr[:, m:m + 1])
            HC = N // 2
            for h in range(2):
                hs = slice(h * HC, (h + 1) * HC)
                ev = e[:, m, hs].rearrange("p (a b) -> p a b", b=CB)
                rr_b = rr_row[:, None, :].to_broadcast([P, HC // CB, CB])
                nc.vector.tensor_mul(ev, ev, rr_b)
                nc.gpsimd.dma_start(out_d.ap()[m][:, hs], e[:, m, hs])

        # ---- emission; per-engine queue order is what matters ----
        for b in range(NB):
            dma_in(b)
        warmup_pe()

        # phase A: per-block norm chains; normalize one-behind sqrt so DVE
        # doesn't head-block on ACT. All sqrts precede all exps.
        for b in range(NB):
            norm_pre(b)
            block_sqrt(b)
            if b > 0:
                block_normalize(b - 1)
        block_normalize(NB - 1)

        # phase B: energy + exp, m-major; tails right after each row tile
        for m in range(MT):
            for g in range(NG):
                pt = energy_group(m, g)
                exp_group(m, g, pt)
            tail(m)

    nc.compile()
    return nc


def kernel(**inputs) -> np.ndarray:
    global _built, LAST_RESULT
    import ml_dtypes

    x = np.asarray(inputs["x"], dtype=np.float32)
    C, W, H = x.shape
    N = W * H
    P = _P
    KO = C // P
    CB = 512
    NB = N // CB
    MT = (N // _NCORES) // P

    if _built is None or _built[1:] != (C, N):
        _built = (_build(C, N), C, N)
    nc = _built[0]

    from concourse import bass_utils

    # block-major fp8 layout: xin[b, p, ko, c] = x[ko*128+p, b*512+c]
    x2 = x.reshape(KO, P, NB, CB)
    xin = np.ascontiguousarray(
        x2.transpose(2, 1, 0, 3)).astype(ml_dtypes.float8_e4m3)

    in_maps = [
        {"x": np.ascontiguousarray(np.roll(xin, -c, axis=0))}
        for c in range(_NCORES)
    ]
    kwargs = {}
    if TRACE:
        kwargs["trace"] = True
        if TRACE_CORES is not None:
            kwargs["trace_cores"] = list(TRACE_CORES)
    res = bass_utils.run_bass_kernel_spmd(
        nc, in_maps, core_ids=list(range(_NCORES)), **kwargs
    )
    LAST_RESULT = res
    out = np.empty((N, N), dtype=np.float32)
    for c in range(_NCORES):
        oc = np.asarray(res.results[c]["out"]).astype(np.float32)
        oc = oc.reshape(MT * P, N)          # rows of this core, rotated cols
        out[c * MT * P:(c + 1) * MT * P] = np.roll(oc, c * CB, axis=1)
    return out.reshape(1, N, N)
